# revision 20
# baseline (speedup 1.0000x reference)
"""Trainium2 Bass kernel: Anscombe transform -> 3x3 Gaussian blur -> inverse
Anscombe, on a [1,4096,4096,3] fp32 image, sharded over H across 8 NeuronCores.

I/O is fp16 on the wire (host casts before upload / after download).

Per core (512 output rows): 4 blocks of 126 rows + one folded 8-row runt.

Main blocks:
  DMA in (3 column pieces of 4096, 8KB descriptors, alternating the two
  HWDGE queues; prefetched one block ahead)
  -> ACT: at = sqrt(4x + 1.5) in 4096-col pieces, software-pipelined into
     the PREVIOUS block's group loop so ACT never stalls the DVE (pad rows
     hold -0.375 so at = 0, matching the reference's zero padding)
  -> PE: full 3x3 conv as 3 accumulated fp16 matmuls per 512-col PSUM chunk
     (vertical taps via banded weight matrix over partitions, horizontal via
     +-3-column shifts of the interleaved-channel rhs).  Weights carry a
     global scale s = c^(-1/3) so PSUM holds ps = s*y, which makes the
     inverse-Anscombe cubic MONIC in r = 1/ps:
        a/y + b/y^2 + c/y^3 = r*(r - ka)*(r - kb)        (exactly)
     with ka,kb = (real roots of c t^2 + b t + a) / s.
  -> ACT: u = Square(0.5/s * ps) = 0.25*y^2
  -> DVE: r = reciprocal_approx_fast(ps)
  -> DVE: custom op  out = r*(r-ka)*(r-kb) + (u - 0.125)   (6 of 8 stages)
  -> DMA out (full-width rows, 24KB descriptors, gpsimd SWDGE).

Runt (8 rows x 12288 cols): folded to [120, 1030] / [96, 1024] tiles
(12 column-groups x 10 input rows on partitions) via rearranged DMA access
patterns, so its elementwise passes cost 1024 columns instead of 12288.
DVE/ACT/PE tile cost is free-size * cycle regardless of partition count, so
the unfolded runt wasted ~26us of DVE alone.
"""

import numpy as np
import ml_dtypes

import concourse.bass as bass
import concourse.bacc as bacc
import concourse.mybir as mybir
import concourse.tile as tile
from concourse import dve_ops
from concourse.bass_utils import run_bass_kernel_spmd
from concourse.dve_spec import C0, C1, C2, Spec, Src0, Src1, _has_src1
from concourse.dve_spec import lower as dve_lower
from concourse.dve_uop import DveOpSpec

F32 = mybir.dt.float32
FP16 = mybir.dt.float16

# ---------------------------------------------------------------- constants
H, W, CH = 4096, 4096, 3
WC = W * CH
N_CORES = 8
H_CORE = H // N_CORES          # output rows per core
BLOCK = 126                    # output rows per full block (128 input rows)
CHUNK = 512                    # matmul N (one PSUM bank)
GROUP = 2048                   # postprocess tile width (4 PSUM banks)
PIECE = 4096                   # input-DMA / sqrt column piece
HB = 63                        # output-DMA row split (two queues per block)
PAD_VAL = -0.375               # sqrt affine maps this to exactly 0
SQRT_SCALE = 4.0               # at = sqrt(4x + 1.5) = 2*sqrt(x + 0.375)
SQRT_BIAS = 1.5

# runt folding: 8 output rows x 12288 cols -> 12 col-groups of 1024 on
# partitions p = 10*cg + row (input) / 8*cg + row (output)
R_M = H_CORE - 4 * BLOCK       # 8 runt output rows
R_KIN = R_M + 2                # 10 input rows
R_NG = 12                      # column groups
R_COLS = WC // R_NG            # 1024
R_PIN = R_NG * R_KIN           # 120 input partitions
R_POUT = R_NG * R_M            # 96 output partitions

# Gaussian kernel exactly as the reference builds it (fp32 throughout)
_coords = np.arange(-1, 2, dtype=np.float32)
_g = np.exp(-(_coords[:, None] ** 2 + _coords[None, :] ** 2)
            / (np.float32(2.0) * np.float32(1.3) ** 2)).astype(np.float32)
K2D = (_g / _g.sum()).astype(np.float32)       # [3,3], rows=dy, cols=dx

_s15 = np.sqrt(np.float64(1.5))
A_C = float(0.25 * _s15)            # coefficient of 1/y
B_C = float(-11.0 / 8.0)            # coefficient of 1/y^2
C_C = float(0.625 * _s15)           # coefficient of 1/y^3
S_PS = float(C_C ** (-1.0 / 3.0))   # PSUM = S_PS * y  (makes the cubic monic)
SQ_SCALE = float(0.5 / S_PS)        # Square(SQ_SCALE*ps) = 0.25*y^2
_disc = float(np.sqrt(B_C * B_C - 4.0 * C_C * A_C))
KA = float((-B_C + _disc) / (2.0 * C_C) / S_PS)
KB = float((-B_C - _disc) / (2.0 * C_C) / S_PS)


# ------------------------------------------------- custom DVE op (the tail)
def _register_tail_op():
    """out = Src0*(Src0-C0)*(Src0-C1) + (Src1 + C2); Src0=r, Src1=u.

    With r = 1/(s*y), u = 0.25*y^2, C0=ka, C1=kb, C2=-1/8 this is exactly
    0.25 y^2 - 0.125 + a/y + b/y^2 + c/y^3   (6 ALU stages of 8)."""
    name = "ANSCOMBE_TAIL_FACT_ANT"
    for op in dve_ops.OPS:
        if op.name == name:
            return op
    spec = Spec(
        body=Src0 * ((Src0 - C0) * (Src0 - C1)) + (Src1 + C2),
        reference=lambda in0, in1, c0, c1, c2: (
            in0.astype(np.float32)
            * ((in0 - np.float32(c0)) * (in0 - np.float32(c1)))
            + (in1.astype(np.float32) + np.float32(c2))
        ).astype(np.float32),
    )
    row = max(dve_ops._SUB_OPCODE_FOR_NAME.values()) + 1
    assert row < 0x20
    dve_ops._SUB_OPCODE_FOR_NAME[name] = row
    shas = {}
    for ver in ("v3", "v4"):
        ds = DveOpSpec(name=name, opcode=row, uops=dve_lower(spec, ver=ver),
                       rd1_en=_has_src1(spec))
        shas[ver] = ds.sha(ver)
    op = dve_ops.DveOp(name, spec, subdim=False, uops_sha=shas)
    dve_ops.OPS.append(op)
    dve_ops.CUSTOM_DVE_SPECS[name] = spec
    return op


def _weight_matrix():
    """[128, 2*BLOCK] fp16 band matrix.  The Gaussian is horizontally
    symmetric (K2D[:,0] == K2D[:,2]), so the +-1-pixel taps share weights:
    GpSimd precomputes t = at(dx=-1) + at(dx=+1) and the PE runs only two
    accumulated matmuls per chunk — segment 0 (vals K2D[d,0]*S_PS) against t,
    segment 1 (vals K2D[d,1]*S_PS) against the center columns."""
    w = K2D.astype(np.float64) * S_PS   # [d, j]
    wm = np.zeros((128, 2 * BLOCK), dtype=np.float64)
    for seg, j in ((0, 0), (1, 1)):
        for d in range(3):
            for m in range(BLOCK):
                wm[m + d, seg * BLOCK + m] = w[d, j]
    return wm.astype(np.float16)


def _runt_weight_matrix():
    """[120, 2*96] fp16 block-banded matrix for the folded runt: out partition
    8*cg + m gets tap d from input partition 10*cg + m + d."""
    w = K2D.astype(np.float64) * S_PS
    wm = np.zeros((R_PIN, 2 * R_POUT), dtype=np.float64)
    for seg, j in ((0, 0), (1, 1)):
        for cg in range(R_NG):
            for d in range(3):
                for m in range(R_M):
                    wm[R_KIN * cg + m + d, seg * R_POUT + R_M * cg + m] = w[d, j]
    return wm.astype(np.float16)


# ------------------------------------------------------------- bass program
def build_nc(h_out=H_CORE, wc=WC):
    tail_op = _register_tail_op()
    h_in = h_out + 2
    nc = bacc.Bacc(None, target_bir_lowering=False)
    # const AP for the sqrt bias (activation converts float bias to an AP)
    _bias = nc.alloc_sbuf_tensor("const-sqrt-bias", [128, 1], F32)
    nc.gpsimd.memset(_bias.ap(), SQRT_BIAS)
    nc.const_aps.aps[(F32, SQRT_BIAS)] = _bias.ap()
    nc.all_engine_barrier()

    x = nc.declare_dram_parameter("x", [h_in, wc], FP16, isOutput=False)
    wmat = nc.declare_dram_parameter("wm", [128, 2 * BLOCK], FP16, isOutput=False)
    wmat2 = nc.declare_dram_parameter("wm2", [R_PIN, 2 * R_POUT], FP16,
                                      isOutput=False)
    out = nc.declare_dram_parameter("out", [h_out, wc], FP16, isOutput=True)

    n_blk = 4                       # full blocks; then the folded runt
    r0_runt = n_blk * BLOCK
    n_grp = wc // GROUP
    n_pc = wc // PIECE
    SQRT = mybir.ActivationFunctionType.Sqrt
    SQUARE = mybir.ActivationFunctionType.Square

    with tile.TileContext(nc) as tc:
        with (
            tc.tile_pool(name="consts", bufs=1) as cpool,
            tc.tile_pool(name="xpool", bufs=2) as xpool,
            tc.tile_pool(name="at", bufs=2) as atpool,
            tc.tile_pool(name="runt", bufs=1) as runtpool,
            tc.tile_pool(name="tpool", bufs=2) as tpool,
            tc.tile_pool(name="upool", bufs=2) as upool,
            tc.tile_pool(name="rpool", bufs=2) as rpool,
            tc.tile_pool(name="opool", bufs=2) as opool,
            tc.tile_pool(name="psum", bufs=2, space="PSUM") as pspool,
        ):
            wt = cpool.tile([128, 2 * BLOCK], FP16)
            wt2 = cpool.tile([R_PIN, 2 * R_POUT], FP16)

            # block 0 uses small leading pieces so the first matmul group's
            # dependencies land ASAP; later blocks prefetch a whole block
            # ahead, so three even pieces suffice.
            PIECES0 = [0, GROUP + 6, 2 * GROUP + 6, 3 * GROUP + 6, wc]
            PIECES = [0, PIECE, 2 * PIECE, wc]

            def issue_main_input(bi):
                """DMA (column pieces, alternating queues) + border memsets."""
                r0 = bi * BLOCK
                k_in = BLOCK + 2
                xc = xpool.tile([128, wc], FP16, tag="xc")
                at = atpool.tile([128, wc + 6], FP16, tag="at")
                bounds = PIECES0 if bi == 0 else PIECES
                for k in range(len(bounds) - 1):
                    c0, c1 = bounds[k], bounds[k + 1]
                    eng = nc.sync if (bi + k) % 2 == 0 else nc.scalar
                    eng.dma_start(xc[:k_in, c0:c1], x[r0:r0 + k_in, c0:c1])
                nc.gpsimd.memset(at[:k_in, 0:3], 0.0)
                nc.gpsimd.memset(at[:k_in, wc + 3:wc + 6], 0.0)
                return xc, at

            def sqrt_piece(tiles, bi, k):
                xc, at = tiles
                bounds = PIECES0 if bi == 0 else PIECES
                c0, c1 = bounds[k], bounds[k + 1]
                nc.scalar.activation(at[:BLOCK + 2, 3 + c0:3 + c1],
                                     xc[:BLOCK + 2, c0:c1],
                                     SQRT, bias=SQRT_BIAS, scale=SQRT_SCALE)

            def issue_runt_input():
                """Folded runt input: [120, 1030] = 12 col-groups x 10 rows,
                3-col halos between groups; PAD_VAL in the outermost borders
                so sqrt maps them to exactly 0 (horizontal zero padding)."""
                x2 = runtpool.tile([R_PIN, R_COLS + 6], FP16, tag="x2")
                at2 = runtpool.tile([R_PIN, R_COLS + 6], FP16, tag="at2")
                nc.gpsimd.memset(x2[:, 0:3], PAD_VAL)
                nc.gpsimd.memset(x2[:, R_COLS + 3:R_COLS + 6], PAD_VAL)
                nc.scalar.dma_start(
                    x2[:, 3:3 + R_COLS],
                    x[r0_runt:r0_runt + R_KIN, :].rearrange(
                        "r (g c) -> g r c", g=R_NG))
                span = (R_NG - 1) * R_COLS
                nc.sync.dma_start(
                    x2[R_KIN:, 0:3],
                    x[r0_runt:r0_runt + R_KIN,
                      R_COLS - 3:R_COLS - 3 + span].rearrange(
                        "r (g c) -> g r c", g=R_NG - 1)[:, :, 0:3])
                nc.sync.dma_start(
                    x2[:(R_NG - 1) * R_KIN, R_COLS + 3:R_COLS + 6],
                    x[r0_runt:r0_runt + R_KIN,
                      R_COLS:R_COLS + span].rearrange(
                        "r (g c) -> g r c", g=R_NG - 1)[:, :, 0:3])
                return x2, at2

            def postprocess(ps, o_ap, m, width):
                u = upool.tile([BLOCK, GROUP], F32, tag="u")
                r = rpool.tile([BLOCK, GROUP], F32, tag="r")
                nc.vector.reciprocal_approx_fast(out=r[:m, :width],
                                                 in_=ps[:m, :width])
                nc.scalar.activation(u[:m, :width], ps[:m, :width],
                                     SQUARE, scale=SQ_SCALE)
                nc.vector._custom_dve(tail_op, out=o_ap,
                                      in0=r[:m, :width], in1=u[:m, :width],
                                      s0=KA, s1=KB, imm2=-0.125)

            def t_add(at, k_in, g0, width):
                """GpSimd side-tap pass: t[n] = at[g0+n] + at[g0+n+6]."""
                t = tpool.tile([128, GROUP], FP16, tag="t")
                nc.gpsimd.tensor_add(t[:k_in, :width],
                                     at[:k_in, g0:g0 + width],
                                     at[:k_in, g0 + 6:g0 + 6 + width])
                return t

            def conv_group(ps, wtile, kp, mout, at, t, g0, width):
                """2 accumulated matmuls per 512-chunk: side taps (t) then
                center; taps outer so consecutive matmuls share weights."""
                for seg, rhs, off in ((0, t, -g0), (1, at, 3)):
                    for c0 in range(g0, g0 + width, CHUNK):
                        cw = min(CHUNK, g0 + width - c0)
                        nc.tensor.matmul(
                            ps[:mout, c0 - g0:c0 - g0 + cw],
                            wtile[:kp, seg * mout:(seg + 1) * mout],
                            rhs[:kp, c0 + off:c0 + off + cw],
                            start=(seg == 0), stop=(seg == 1),
                        )

            def runt_mm():
                """Folded runt conv into a psum tile; postprocess must follow
                within one pool rotation (caller emits it right after the
                surrounding group's postprocess)."""
                _, at2 = tiles[n_blk]
                t2 = runtpool.tile([R_PIN, R_COLS], FP16, tag="t2")
                nc.gpsimd.tensor_add(t2[:, :], at2[:, 0:R_COLS],
                                     at2[:, 6:6 + R_COLS])
                ps = pspool.tile([BLOCK, GROUP], F32, tag="ps")
                for seg, rhs, off in ((0, t2, 0), (1, at2, 3)):
                    for c0 in range(0, R_COLS, CHUNK):
                        nc.tensor.matmul(
                            ps[:R_POUT, c0:c0 + CHUNK],
                            wt2[:R_PIN, seg * R_POUT:(seg + 1) * R_POUT],
                            rhs[:R_PIN, c0 + off:c0 + off + CHUNK],
                            start=(seg == 0), stop=(seg == 1),
                        )
                return ps

            def runt_post(ps):
                o2 = opool.tile([BLOCK, wc], FP16, tag="o")
                postprocess(ps, o2[:R_POUT, :R_COLS], R_POUT, R_COLS)
                # SWDGE only: the 3-level DRAM dest pattern exceeds PDMA2D
                nc.gpsimd.dma_start(
                    out[r0_runt:r0_runt + R_M, :].rearrange(
                        "r (g c) -> g r c", g=R_NG),
                    o2[:R_POUT, :R_COLS])

            # block 0 leads with a 512-col group so the first PSUM tile (and
            # the DVE) is live as early as possible
            GB0 = [0, CHUNK, GROUP] + [g * GROUP for g in range(2, n_grp + 1)]
            GB = [g * GROUP for g in range(n_grp + 1)]

            # ---- prime the pipeline (weights lead the scalar queue: the
            # first matmul needs them; block-0 piece 0 leads the sync queue)
            tiles = [None] * (n_blk + 1)
            nc.scalar.dma_start(wt[:], wmat[:])
            tiles[0] = issue_main_input(0)
            nc.scalar.dma_start(wt2[:], wmat2[:])
            sqrt_piece(tiles[0], 0, 0)
            if n_blk > 1:
                tiles[1] = issue_main_input(1)

            # ---- main blocks (tcur = side-tap tile, computed one group ahead)
            k_in = BLOCK + 2
            tcur = t_add(tiles[0][1], k_in, GB0[0], GB0[1] - GB0[0])
            for bi in range(n_blk):
                r0 = bi * BLOCK
                xc, at = tiles[bi]
                last = bi == n_blk - 1
                bounds = GB0 if bi == 0 else GB
                ng = len(bounds) - 1
                o = opool.tile([BLOCK, wc], FP16, tag="o")
                if bi == 2:
                    tiles[n_blk] = issue_runt_input()
                for g in range(ng):
                    g0, g1 = bounds[g], bounds[g + 1]
                    width = g1 - g0
                    if g + 1 < ng:
                        tnxt = t_add(at, k_in, bounds[g + 1],
                                     bounds[g + 2] - bounds[g + 1])
                    elif not last:
                        tnxt = t_add(tiles[bi + 1][1], k_in, GB[0],
                                     GB[1] - GB[0])
                    ps = pspool.tile([BLOCK, GROUP], F32, tag="ps")
                    conv_group(ps, wt, k_in, BLOCK, at, tcur, g0, width)
                    tcur = tnxt
                    if last and g == 0:
                        rps = runt_mm()
                    postprocess(ps, o[:BLOCK, g0:g1], BLOCK, width)
                    if last and g == 0:
                        runt_post(rps)
                    # software-pipelined ACT prep: finish this block's own
                    # remaining sqrt pieces first (block 0), then the next
                    # block's, so the ACT queue never delays a Square long.
                    if bi == 0 and g < 3:
                        sqrt_piece(tiles[0], 0, g + 1)
                    elif bi == 0 and g >= 4 and n_blk > 1:
                        sqrt_piece(tiles[1], 1, g - 4)
                    elif 0 < bi < n_blk - 1 and g % 2 == 0:
                        sqrt_piece(tiles[bi + 1], bi + 1, g // 2)
                    elif bi == n_blk - 2 and g == 5:
                        x2, at2 = tiles[n_blk]
                        nc.scalar.activation(at2[:, :], x2[:, :], SQRT,
                                             bias=SQRT_BIAS, scale=SQRT_SCALE)
                    if last and g == 2:
                        # block-3 cols 0:6144 -> DRAM early, split two queues
                        nc.sync.dma_start(out[r0:r0 + HB, :wc // 2],
                                          o[:HB, :wc // 2])
                        nc.scalar.dma_start(out[r0 + HB:r0 + BLOCK, :wc // 2],
                                            o[HB:BLOCK, :wc // 2])
                    if last and g == 4:
                        c0, c1 = wc // 2, wc - GROUP
                        nc.sync.dma_start(out[r0:r0 + HB, c0:c1],
                                          o[:HB, c0:c1])
                        nc.scalar.dma_start(out[r0 + HB:r0 + BLOCK, c0:c1],
                                            o[HB:BLOCK, c0:c1])
                if not last:
                    # full-width output rows (24KB descriptors), two HWDGE
                    # queues; GpSimd is busy with the side-tap adds now
                    nc.sync.dma_start(out[r0:r0 + HB, :], o[:HB, :])
                    nc.scalar.dma_start(out[r0 + HB:r0 + BLOCK, :],
                                        o[HB:BLOCK, :])
                else:
                    # only the last group remains after the final tail
                    c0 = wc - GROUP
                    nc.sync.dma_start(out[r0:r0 + HB, c0:], o[:HB, c0:])
                    nc.scalar.dma_start(out[r0 + HB:r0 + BLOCK, c0:],
                                        o[HB:BLOCK, c0:])
                if bi + 2 < n_blk:
                    tiles[bi + 2] = issue_main_input(bi + 2)
    nc.compile()
    return nc


# ------------------------------------------------------------------- driver
_CACHE = {}


def _get_nc(h_out, wc):
    key = (h_out, wc)
    if key not in _CACHE:
        _CACHE[key] = build_nc(h_out, wc)
    return _CACHE[key]


def run_sharded(x2d, n_cores=N_CORES, trace=False, **kw):
    """x2d: [H, W*C] fp32 full image (2D). Returns ([H, W*C] fp32, results)."""
    h, wc = x2d.shape
    h_core = h // n_cores
    nc = _get_nc(h_core, wc)
    wm = _weight_matrix()
    wm2 = _runt_weight_matrix()
    in_maps = []
    for i in range(n_cores):
        lo, hi = i * h_core - 1, (i + 1) * h_core + 1
        src_lo, src_hi = max(lo, 0), min(hi, h)
        if lo < 0 or hi > h:
            slab = np.full((h_core + 2, wc), PAD_VAL, dtype=np.float16)
        else:
            slab = np.empty((h_core + 2, wc), dtype=np.float16)
        slab[src_lo - lo:src_hi - lo] = x2d[src_lo:src_hi]
        in_maps.append({"x": slab, "wm": wm, "wm2": wm2})
    res = run_bass_kernel_spmd(nc, in_maps, list(range(n_cores)), trace=trace, **kw)
    full = np.concatenate([res.results[i]["out"] for i in range(n_cores)],
                          axis=0).astype(np.float32)
    return full, res


def kernel(im: np.ndarray) -> np.ndarray:
    x2d = np.asarray(im, dtype=np.float32).reshape(H, WC)
    full, _ = run_sharded(x2d)
    return full.reshape(H, W, CH)


# revision 22
# speedup vs baseline: 1.2628x; 1.2628x over previous
"""Trainium2 Bass kernel: Anscombe transform -> 3x3 Gaussian blur -> inverse
Anscombe, on a [1,4096,4096,3] fp32 image, sharded over H across 8 NeuronCores.

I/O is fp16 on the wire (host casts before upload / after download).

Per core (512 output rows): 4 blocks of 126 rows + one folded 8-row runt.

Main blocks:
  DMA in (3 column pieces of 4096, 8KB descriptors, alternating the two
  HWDGE queues; prefetched one block ahead)
  -> ACT: at = sqrt(4x + 1.5) in 4096-col pieces, software-pipelined into
     the PREVIOUS block's group loop so ACT never stalls the DVE (pad rows
     hold -0.375 so at = 0, matching the reference's zero padding)
  -> PE: full 3x3 conv as 3 accumulated fp16 matmuls per 512-col PSUM chunk
     (vertical taps via banded weight matrix over partitions, horizontal via
     +-3-column shifts of the interleaved-channel rhs).  Weights carry a
     global scale s = c^(-1/3) so PSUM holds ps = s*y, which makes the
     inverse-Anscombe cubic MONIC in r = 1/ps:
        a/y + b/y^2 + c/y^3 = r*(r - ka)*(r - kb)        (exactly)
     with ka,kb = (real roots of c t^2 + b t + a) / s.
  -> ACT: u = Square(0.5/s * ps) = 0.25*y^2
  -> DVE: r = reciprocal_approx_fast(ps)
  -> DVE: custom op  out = r*(r-ka)*(r-kb) + (u - 0.125)   (6 of 8 stages)
  -> DMA out (full-width rows, 24KB descriptors, gpsimd SWDGE).

Runt (8 rows x 12288 cols): folded to [120, 1030] / [96, 1024] tiles
(12 column-groups x 10 input rows on partitions) via rearranged DMA access
patterns, so its elementwise passes cost 1024 columns instead of 12288.
DVE/ACT/PE tile cost is free-size * cycle regardless of partition count, so
the unfolded runt wasted ~26us of DVE alone.
"""

import numpy as np
import ml_dtypes

import concourse.bass as bass
import concourse.bacc as bacc
import concourse.mybir as mybir
import concourse.tile as tile
from concourse import dve_ops
from concourse.bass_utils import run_bass_kernel_spmd
from concourse.dve_spec import AluOp, Bin, C0, C1, C2, Spec, Src0, Src1, _has_src1
from concourse.dve_spec import lower as dve_lower
from concourse.dve_uop import DveOpSpec

F32 = mybir.dt.float32
FP16 = mybir.dt.float16

# ---------------------------------------------------------------- constants
H, W, CH = 4096, 4096, 3
WC = W * CH
N_CORES = 8
H_CORE = H // N_CORES          # output rows per core
BLOCK = 126                    # output rows per full block (128 input rows)
CHUNK = 512                    # matmul N (one PSUM bank)
GROUP = 2048                   # postprocess tile width (4 PSUM banks)
PIECE = 4096                   # input-DMA / sqrt column piece
HB = 63                        # output-DMA row split (two queues per block)
PAD_VAL = -0.375               # sqrt affine maps this to exactly 0
SQRT_SCALE = 4.0               # at = sqrt(4x + 1.5) = 2*sqrt(x + 0.375)
SQRT_BIAS = 1.5

# runt folding: 8 output rows x 12288 cols -> 12 col-groups of 1024 on
# partitions p = 10*cg + row (input) / 8*cg + row (output)
R_M = H_CORE - 4 * BLOCK       # 8 runt output rows
R_KIN = R_M + 2                # 10 input rows
R_NG = 12                      # column groups
R_COLS = WC // R_NG            # 1024
R_PIN = R_NG * R_KIN           # 120 input partitions
R_POUT = R_NG * R_M            # 96 output partitions

# Gaussian kernel exactly as the reference builds it (fp32 throughout)
_coords = np.arange(-1, 2, dtype=np.float32)
_g = np.exp(-(_coords[:, None] ** 2 + _coords[None, :] ** 2)
            / (np.float32(2.0) * np.float32(1.3) ** 2)).astype(np.float32)
K2D = (_g / _g.sum()).astype(np.float32)       # [3,3], rows=dy, cols=dx

_s15 = np.sqrt(np.float64(1.5))
A_C = float(0.25 * _s15)            # coefficient of 1/y
B_C = float(-11.0 / 8.0)            # coefficient of 1/y^2
C_C = float(0.625 * _s15)           # coefficient of 1/y^3
S_PS = 0.5                          # PSUM = y/2: (ps-KQ)(ps+KQ) = y^2/4 - 1/8
K3 = float(C_C ** (1.0 / 3.0) / 2)  # recip scale: r = K3/ps -> monic cubic
KQ = float(np.sqrt(0.125))
_disc = float(np.sqrt(B_C * B_C - 4.0 * C_C * A_C))
KA = float(2 * K3 * (-B_C + _disc) / (2.0 * C_C))
KB = float(2 * K3 * (-B_C - _disc) / (2.0 * C_C))
# Chebyshev seed constants of the BITWISE_NOT reciprocal (same pair as
# concourse's RECIPROCAL_APPROX_FAST; minimax-optimal for one NR pass too,
# ~1.7e-3 rel err -- the 0.25y^2 term rides exactly, only 1/y^k terms see it)
RC0, RC1 = -0.23549792, 2.0017324


# ------------------------------------------------ custom DVE ops (postproc)
def _register_op(name, spec):
    for op in dve_ops.OPS:
        if op.name == name:
            return op
    row = max(dve_ops._SUB_OPCODE_FOR_NAME.values()) + 1
    assert row < 0x20
    dve_ops._SUB_OPCODE_FOR_NAME[name] = row
    shas = {}
    for ver in ("v3", "v4"):
        ds = DveOpSpec(name=name, opcode=row, uops=dve_lower(spec, ver=ver),
                       rd1_en=_has_src1(spec))
        shas[ver] = ds.sha(ver)
    op = dve_ops.DveOp(name, spec, subdim=False, uops_sha=shas)
    dve_ops.OPS.append(op)
    dve_ops.CUSTOM_DVE_SPECS[name] = spec
    return op


def _ref_recip1s(in0, in1, c0, c1, c2):
    notx = (~in0.view(np.int32)).view(np.float32)
    y0 = (np.float32(c0) * notx).astype(np.float32)
    y1 = (y0 * (np.float32(c1) - in0 * y0)).astype(np.float32)
    return (y1 * np.float32(c2)).astype(np.float32)


def _register_recip1s_op():
    """r = C2 / Src0 (approx): BITWISE_NOT exponent-flip seed + one NR pass
    + final scale; 6 of 8 stages, ~1.7e-3 rel err."""
    _n = Bin(AluOp.BITWISE_NOT, Src0, Src0)
    _y0 = _n * C0
    _y1 = _y0 * (C1 - Src0 * _y0)
    return _register_op("ANSCOMBE_RECIP1S_ANT",
                        Spec(body=_y1 * C2, reference=_ref_recip1s))


def _register_tail_op():
    """out = Src0*(Src0-C0)*(Src0-C1) + (Src1-C2)*(Src1+C2); Src0 = r =
    K3/ps, Src1 = ps = y/2.  With C0=KA, C1=KB, C2=KQ this is exactly
    0.25 y^2 - 0.125 + a/y + b/y^2 + c/y^3   (8 ALU stages of 8)."""
    spec = Spec(
        body=Src0 * ((Src0 - C0) * (Src0 - C1)) + (Src1 - C2) * (Src1 + C2),
        reference=lambda in0, in1, c0, c1, c2: (
            in0.astype(np.float32)
            * ((in0 - np.float32(c0)) * (in0 - np.float32(c1)))
            + (in1 - np.float32(c2)) * (in1 + np.float32(c2))
        ).astype(np.float32),
    )
    return _register_op("ANSCOMBE_TAIL_FACT2_ANT", spec)


def _weight_matrix():
    """[128, 3*BLOCK] fp16 band matrix: segment j (horizontal tap dx=j-1) has
    K2D[d, j]*S_PS on diagonal k-m = d (vertical tap dy=d-1)."""
    w = K2D.astype(np.float64) * S_PS   # [d, j]
    wm = np.zeros((128, 3 * BLOCK), dtype=np.float64)
    for j in range(3):
        for d in range(3):
            for m in range(BLOCK):
                wm[m + d, j * BLOCK + m] = w[d, j]
    return wm.astype(np.float16)


def _runt_weight_matrix():
    """[120, 3*96] fp16 block-banded matrix for the folded runt: out partition
    8*cg + m gets tap d from input partition 10*cg + m + d."""
    w = K2D.astype(np.float64) * S_PS
    wm = np.zeros((R_PIN, 3 * R_POUT), dtype=np.float64)
    for j in range(3):
        for cg in range(R_NG):
            for d in range(3):
                for m in range(R_M):
                    wm[R_KIN * cg + m + d, j * R_POUT + R_M * cg + m] = w[d, j]
    return wm.astype(np.float16)


# ------------------------------------------------------------- bass program
def build_nc(h_out=H_CORE, wc=WC):
    tail_op = _register_tail_op()
    recip_op = _register_recip1s_op()
    h_in = h_out + 2
    nc = bacc.Bacc(None, target_bir_lowering=False)
    # const AP for the sqrt bias (activation converts float bias to an AP)
    _bias = nc.alloc_sbuf_tensor("const-sqrt-bias", [128, 1], F32)
    nc.gpsimd.memset(_bias.ap(), SQRT_BIAS)
    nc.const_aps.aps[(F32, SQRT_BIAS)] = _bias.ap()
    nc.all_engine_barrier()

    x = nc.declare_dram_parameter("x", [h_in, wc], FP16, isOutput=False)
    wmat = nc.declare_dram_parameter("wm", [128, 3 * BLOCK], FP16, isOutput=False)
    wmat2 = nc.declare_dram_parameter("wm2", [R_PIN, 3 * R_POUT], FP16,
                                      isOutput=False)
    out = nc.declare_dram_parameter("out", [h_out, wc], FP16, isOutput=True)

    n_blk = 4                       # full blocks; then the folded runt
    r0_runt = n_blk * BLOCK
    n_grp = wc // GROUP
    n_pc = wc // PIECE
    SQRT = mybir.ActivationFunctionType.Sqrt
    SQUARE = mybir.ActivationFunctionType.Square

    with tile.TileContext(nc) as tc:
        with (
            tc.tile_pool(name="consts", bufs=1) as cpool,
            tc.tile_pool(name="xpool", bufs=2) as xpool,
            tc.tile_pool(name="at", bufs=2) as atpool,
            tc.tile_pool(name="runt", bufs=1) as runtpool,
            tc.tile_pool(name="rpool", bufs=2) as rpool,
            tc.tile_pool(name="opool", bufs=2) as opool,
            tc.tile_pool(name="psum", bufs=2, space="PSUM") as pspool,
        ):
            wt = cpool.tile([128, 3 * BLOCK], FP16)
            wt2 = cpool.tile([R_PIN, 3 * R_POUT], FP16)

            # block 0 uses small leading pieces so the first matmul group's
            # dependencies land ASAP; later blocks prefetch a whole block
            # ahead, so three even pieces suffice.
            PIECES0 = [0, GROUP + 6, 2 * GROUP + 6, 3 * GROUP + 6, wc]
            PIECES = [0, PIECE, 2 * PIECE, wc]

            def issue_main_input(bi):
                """DMA (column pieces, alternating queues) + border memsets."""
                r0 = bi * BLOCK
                k_in = BLOCK + 2
                xc = xpool.tile([128, wc], FP16, tag="xc")
                at = atpool.tile([128, wc + 6], FP16, tag="at")
                bounds = PIECES0 if bi == 0 else PIECES
                for k in range(len(bounds) - 1):
                    c0, c1 = bounds[k], bounds[k + 1]
                    eng = nc.sync if (bi + k) % 2 == 0 else nc.scalar
                    eng.dma_start(xc[:k_in, c0:c1], x[r0:r0 + k_in, c0:c1])
                nc.gpsimd.memset(at[:k_in, 0:3], 0.0)
                nc.gpsimd.memset(at[:k_in, wc + 3:wc + 6], 0.0)
                return xc, at

            def sqrt_piece(tiles, bi, k):
                xc, at = tiles
                bounds = PIECES0 if bi == 0 else PIECES
                c0, c1 = bounds[k], bounds[k + 1]
                nc.scalar.activation(at[:BLOCK + 2, 3 + c0:3 + c1],
                                     xc[:BLOCK + 2, c0:c1],
                                     SQRT, bias=SQRT_BIAS, scale=SQRT_SCALE)

            def issue_runt_input():
                """Folded runt input: [120, 1030] = 12 col-groups x 10 rows,
                3-col halos between groups; PAD_VAL in the outermost borders
                so sqrt maps them to exactly 0 (horizontal zero padding)."""
                x2 = runtpool.tile([R_PIN, R_COLS + 6], FP16, tag="x2")
                at2 = runtpool.tile([R_PIN, R_COLS + 6], FP16, tag="at2")
                nc.gpsimd.memset(x2[:, 0:3], PAD_VAL)
                nc.gpsimd.memset(x2[:, R_COLS + 3:R_COLS + 6], PAD_VAL)
                nc.scalar.dma_start(
                    x2[:, 3:3 + R_COLS],
                    x[r0_runt:r0_runt + R_KIN, :].rearrange(
                        "r (g c) -> g r c", g=R_NG))
                span = (R_NG - 1) * R_COLS
                nc.sync.dma_start(
                    x2[R_KIN:, 0:3],
                    x[r0_runt:r0_runt + R_KIN,
                      R_COLS - 3:R_COLS - 3 + span].rearrange(
                        "r (g c) -> g r c", g=R_NG - 1)[:, :, 0:3])
                nc.sync.dma_start(
                    x2[:(R_NG - 1) * R_KIN, R_COLS + 3:R_COLS + 6],
                    x[r0_runt:r0_runt + R_KIN,
                      R_COLS:R_COLS + span].rearrange(
                        "r (g c) -> g r c", g=R_NG - 1)[:, :, 0:3])
                return x2, at2

            def postprocess(ps, o_ap, m, width):
                r = rpool.tile([BLOCK, GROUP], F32, tag="r")
                nc.vector._custom_dve(recip_op, out=r[:m, :width],
                                      in0=ps[:m, :width],
                                      s0=RC0, s1=RC1, imm2=K3)
                nc.vector._custom_dve(tail_op, out=o_ap,
                                      in0=r[:m, :width], in1=ps[:m, :width],
                                      s0=KA, s1=KB, imm2=KQ)

            def runt_mm():
                """Folded runt conv into a psum tile; postprocess must follow
                within one pool rotation (caller emits it right after the
                surrounding group's postprocess)."""
                _, at2 = tiles[n_blk]
                ps = pspool.tile([BLOCK, GROUP], F32, tag="ps")
                for j in range(3):
                    for c0 in range(0, R_COLS, CHUNK):
                        nc.tensor.matmul(
                            ps[:R_POUT, c0:c0 + CHUNK],
                            wt2[:R_PIN, j * R_POUT:(j + 1) * R_POUT],
                            at2[:R_PIN, c0 + 3 * j:c0 + 3 * j + CHUNK],
                            start=(j == 0), stop=(j == 2),
                        )
                return ps

            def runt_post(ps):
                o2 = opool.tile([BLOCK, wc], FP16, tag="o")
                postprocess(ps, o2[:R_POUT, :R_COLS], R_POUT, R_COLS)
                # SWDGE only: the 3-level DRAM dest pattern exceeds PDMA2D
                nc.gpsimd.dma_start(
                    out[r0_runt:r0_runt + R_M, :].rearrange(
                        "r (g c) -> g r c", g=R_NG),
                    o2[:R_POUT, :R_COLS])

            # block 0 leads with a 512-col group so the first PSUM tile (and
            # the DVE) is live as early as possible
            GB0 = [0, CHUNK, GROUP] + [g * GROUP for g in range(2, n_grp + 1)]
            GB = [g * GROUP for g in range(n_grp + 1)]

            # ---- prime the pipeline (weights lead the scalar queue: the
            # first matmul needs them; block-0 piece 0 leads the sync queue)
            tiles = [None] * (n_blk + 1)
            nc.scalar.dma_start(wt[:], wmat[:])
            tiles[0] = issue_main_input(0)
            nc.scalar.dma_start(wt2[:], wmat2[:])
            sqrt_piece(tiles[0], 0, 0)
            if n_blk > 1:
                tiles[1] = issue_main_input(1)

            # ---- main blocks
            for bi in range(n_blk):
                r0 = bi * BLOCK
                k_in = BLOCK + 2
                xc, at = tiles[bi]
                last = bi == n_blk - 1
                bounds = GB0 if bi == 0 else GB
                ng = len(bounds) - 1
                o = opool.tile([BLOCK, wc], FP16, tag="o")
                if bi == 2:
                    tiles[n_blk] = issue_runt_input()
                for g in range(ng):
                    g0, g1 = bounds[g], bounds[g + 1]
                    width = g1 - g0
                    ps = pspool.tile([BLOCK, GROUP], F32, tag="ps")
                    # taps outer so consecutive matmuls share stationary weights
                    for j in range(3):
                        for c0 in range(g0, g1, CHUNK):
                            cw = min(CHUNK, g1 - c0)
                            nc.tensor.matmul(
                                ps[:BLOCK, c0 - g0:c0 - g0 + cw],
                                wt[:k_in, j * BLOCK:(j + 1) * BLOCK],
                                at[:k_in, c0 + 3 * j:c0 + 3 * j + cw],
                                start=(j == 0), stop=(j == 2),
                            )
                    if last and g == 0:
                        rps = runt_mm()
                    postprocess(ps, o[:BLOCK, g0:g1], BLOCK, width)
                    if last and g == 0:
                        runt_post(rps)
                    # software-pipelined ACT prep: finish this block's own
                    # remaining sqrt pieces first (block 0), then the next
                    # block's, so the ACT queue never delays a Square long.
                    if bi == 0 and g < 3:
                        sqrt_piece(tiles[0], 0, g + 1)
                    elif bi == 0 and g >= 4 and n_blk > 1:
                        sqrt_piece(tiles[1], 1, g - 4)
                    elif 0 < bi < n_blk - 1 and g % 2 == 0:
                        sqrt_piece(tiles[bi + 1], bi + 1, g // 2)
                    elif bi == n_blk - 2 and g == 5:
                        x2, at2 = tiles[n_blk]
                        nc.scalar.activation(at2[:, :], x2[:, :], SQRT,
                                             bias=SQRT_BIAS, scale=SQRT_SCALE)
                    if last and g == 2:
                        # block-3 cols 0:6144 -> DRAM early, split two queues
                        nc.gpsimd.dma_start(out[r0:r0 + HB, :wc // 2],
                                            o[:HB, :wc // 2])
                        nc.scalar.dma_start(out[r0 + HB:r0 + BLOCK, :wc // 2],
                                            o[HB:BLOCK, :wc // 2])
                    if last and g == 4:
                        c0, c1 = wc // 2, wc - GROUP
                        nc.gpsimd.dma_start(out[r0:r0 + HB, c0:c1],
                                            o[:HB, c0:c1])
                        nc.sync.dma_start(out[r0 + HB:r0 + BLOCK, c0:c1],
                                          o[HB:BLOCK, c0:c1])
                if not last:
                    # full-width output rows (24KB descriptors), two queues
                    nc.gpsimd.dma_start(out[r0:r0 + HB, :], o[:HB, :])
                    eng = nc.sync if bi % 2 == 0 else nc.scalar
                    eng.dma_start(out[r0 + HB:r0 + BLOCK, :], o[HB:BLOCK, :])
                else:
                    # only the last group remains after the final tail
                    c0 = wc - GROUP
                    nc.gpsimd.dma_start(out[r0:r0 + HB, c0:], o[:HB, c0:])
                    nc.sync.dma_start(out[r0 + HB:r0 + BLOCK, c0:],
                                      o[HB:BLOCK, c0:])
                if bi + 2 < n_blk:
                    tiles[bi + 2] = issue_main_input(bi + 2)
    nc.compile()
    return nc


# ------------------------------------------------------------------- driver
_CACHE = {}


def _get_nc(h_out, wc):
    key = (h_out, wc)
    if key not in _CACHE:
        _CACHE[key] = build_nc(h_out, wc)
    return _CACHE[key]


def run_sharded(x2d, n_cores=N_CORES, trace=False, **kw):
    """x2d: [H, W*C] fp32 full image (2D). Returns ([H, W*C] fp32, results)."""
    h, wc = x2d.shape
    h_core = h // n_cores
    nc = _get_nc(h_core, wc)
    wm = _weight_matrix()
    wm2 = _runt_weight_matrix()
    in_maps = []
    for i in range(n_cores):
        lo, hi = i * h_core - 1, (i + 1) * h_core + 1
        src_lo, src_hi = max(lo, 0), min(hi, h)
        if lo < 0 or hi > h:
            slab = np.full((h_core + 2, wc), PAD_VAL, dtype=np.float16)
        else:
            slab = np.empty((h_core + 2, wc), dtype=np.float16)
        slab[src_lo - lo:src_hi - lo] = x2d[src_lo:src_hi]
        in_maps.append({"x": slab, "wm": wm, "wm2": wm2})
    res = run_bass_kernel_spmd(nc, in_maps, list(range(n_cores)), trace=trace, **kw)
    full = np.concatenate([res.results[i]["out"] for i in range(n_cores)],
                          axis=0).astype(np.float32)
    return full, res


def kernel(im: np.ndarray) -> np.ndarray:
    x2d = np.asarray(im, dtype=np.float32).reshape(H, WC)
    full, _ = run_sharded(x2d)
    return full.reshape(H, W, CH)


# revision 23
# speedup vs baseline: 1.3598x; 1.0768x over previous
"""Trainium2 Bass kernel: Anscombe transform -> 3x3 Gaussian blur -> inverse
Anscombe, on a [1,4096,4096,3] fp32 image, sharded over H across 8 NeuronCores.

I/O is fp16 on the wire (host casts before upload / after download).

Per core (512 output rows): 4 blocks of 126 rows + one folded 8-row runt.

Main blocks:
  DMA in (3 column pieces of 4096, 8KB descriptors, alternating the two
  HWDGE queues; prefetched one block ahead)
  -> ACT: at = sqrt(4x + 1.5) in 4096-col pieces, software-pipelined into
     the PREVIOUS block's group loop so ACT never stalls the DVE (pad rows
     hold -0.375 so at = 0, matching the reference's zero padding)
  -> PE: full 3x3 conv as 3 accumulated fp16 matmuls per 512-col PSUM chunk
     (vertical taps via banded weight matrix over partitions, horizontal via
     +-3-column shifts of the interleaved-channel rhs).  Weights carry a
     global scale s = c^(-1/3) so PSUM holds ps = s*y, which makes the
     inverse-Anscombe cubic MONIC in r = 1/ps:
        a/y + b/y^2 + c/y^3 = r*(r - ka)*(r - kb)        (exactly)
     with ka,kb = (real roots of c t^2 + b t + a) / s.
  -> ACT: u = Square(0.5/s * ps) = 0.25*y^2
  -> DVE: r = reciprocal_approx_fast(ps)
  -> DVE: custom op  out = r*(r-ka)*(r-kb) + (u - 0.125)   (6 of 8 stages)
  -> DMA out (full-width rows, 24KB descriptors, gpsimd SWDGE).

Runt (8 rows x 12288 cols): folded to [120, 1030] / [96, 1024] tiles
(12 column-groups x 10 input rows on partitions) via rearranged DMA access
patterns, so its elementwise passes cost 1024 columns instead of 12288.
DVE/ACT/PE tile cost is free-size * cycle regardless of partition count, so
the unfolded runt wasted ~26us of DVE alone.
"""

import numpy as np
import ml_dtypes

import concourse.bass as bass
import concourse.bacc as bacc
import concourse.mybir as mybir
import concourse.tile as tile
from concourse import dve_ops
from concourse.bass_utils import run_bass_kernel_spmd
from concourse.dve_spec import AluOp, Bin, C0, C1, C2, Spec, Src0, Src1, _has_src1
from concourse.dve_spec import lower as dve_lower
from concourse.dve_uop import DveOpSpec

F32 = mybir.dt.float32
FP16 = mybir.dt.float16

# ---------------------------------------------------------------- constants
H, W, CH = 4096, 4096, 3
WC = W * CH
N_CORES = 8
H_CORE = H // N_CORES          # output rows per core
BLOCK = 126                    # output rows per full block (128 input rows)
CHUNK = 512                    # matmul N (one PSUM bank)
GROUP = 2048                   # postprocess tile width (4 PSUM banks)
PIECE = 4096                   # input-DMA / sqrt column piece
HB = 63                        # output-DMA row split (two queues per block)
PAD_VAL = -0.375               # sqrt affine maps this to exactly 0
SQRT_SCALE = 4.0               # at = sqrt(4x + 1.5) = 2*sqrt(x + 0.375)
SQRT_BIAS = 1.5

# runt folding: 8 output rows x 12288 cols -> 12 col-groups of 1024 on
# partitions p = 10*cg + row (input) / 8*cg + row (output)
R_M = H_CORE - 4 * BLOCK       # 8 runt output rows
R_KIN = R_M + 2                # 10 input rows
R_NG = 12                      # column groups
R_COLS = WC // R_NG            # 1024
R_PIN = R_NG * R_KIN           # 120 input partitions
R_POUT = R_NG * R_M            # 96 output partitions

# Gaussian kernel exactly as the reference builds it (fp32 throughout)
_coords = np.arange(-1, 2, dtype=np.float32)
_g = np.exp(-(_coords[:, None] ** 2 + _coords[None, :] ** 2)
            / (np.float32(2.0) * np.float32(1.3) ** 2)).astype(np.float32)
K2D = (_g / _g.sum()).astype(np.float32)       # [3,3], rows=dy, cols=dx

_s15 = np.sqrt(np.float64(1.5))
A_C = float(0.25 * _s15)            # coefficient of 1/y
B_C = float(-11.0 / 8.0)            # coefficient of 1/y^2
C_C = float(0.625 * _s15)           # coefficient of 1/y^3
S_PS = 0.5                          # PSUM = y/2: (ps-KQ)(ps+KQ) = y^2/4 - 1/8
K3 = float(C_C ** (1.0 / 3.0) / 2)  # recip scale: r = K3/ps -> monic cubic
KQ = float(np.sqrt(0.125))
_disc = float(np.sqrt(B_C * B_C - 4.0 * C_C * A_C))
KA = float(2 * K3 * (-B_C + _disc) / (2.0 * C_C))
KB = float(2 * K3 * (-B_C - _disc) / (2.0 * C_C))
# Chebyshev seed constants of the BITWISE_NOT reciprocal (same pair as
# concourse's RECIPROCAL_APPROX_FAST; minimax-optimal for one NR pass too,
# ~1.7e-3 rel err -- the 0.25y^2 term rides exactly, only 1/y^k terms see it)
RC0, RC1 = -0.23549792, 2.0017324


# ------------------------------------------------ custom DVE ops (postproc)
def _register_op(name, spec):
    for op in dve_ops.OPS:
        if op.name == name:
            return op
    row = max(dve_ops._SUB_OPCODE_FOR_NAME.values()) + 1
    assert row < 0x20
    dve_ops._SUB_OPCODE_FOR_NAME[name] = row
    shas = {}
    for ver in ("v3", "v4"):
        ds = DveOpSpec(name=name, opcode=row, uops=dve_lower(spec, ver=ver),
                       rd1_en=_has_src1(spec))
        shas[ver] = ds.sha(ver)
    op = dve_ops.DveOp(name, spec, subdim=False, uops_sha=shas)
    dve_ops.OPS.append(op)
    dve_ops.CUSTOM_DVE_SPECS[name] = spec
    return op


def _ref_recip1s(in0, in1, c0, c1, c2):
    notx = (~in0.view(np.int32)).view(np.float32)
    y0 = (np.float32(c0) * notx).astype(np.float32)
    y1 = (y0 * (np.float32(c1) - in0 * y0)).astype(np.float32)
    return (y1 * np.float32(c2)).astype(np.float32)


def _register_recip1s_op():
    """r = C2 / Src0 (approx): BITWISE_NOT exponent-flip seed + one NR pass
    + final scale; 6 of 8 stages, ~1.7e-3 rel err."""
    _n = Bin(AluOp.BITWISE_NOT, Src0, Src0)
    _y0 = _n * C0
    _y1 = _y0 * (C1 - Src0 * _y0)
    return _register_op("ANSCOMBE_RECIP1S_ANT",
                        Spec(body=_y1 * C2, reference=_ref_recip1s))


def _register_tail_op():
    """out = Src0*(Src0-C0)*(Src0-C1) + (Src1-C2)*(Src1+C2); Src0 = r =
    K3/ps, Src1 = ps = y/2.  With C0=KA, C1=KB, C2=KQ this is exactly
    0.25 y^2 - 0.125 + a/y + b/y^2 + c/y^3   (8 ALU stages of 8)."""
    spec = Spec(
        body=Src0 * ((Src0 - C0) * (Src0 - C1)) + (Src1 - C2) * (Src1 + C2),
        reference=lambda in0, in1, c0, c1, c2: (
            in0.astype(np.float32)
            * ((in0 - np.float32(c0)) * (in0 - np.float32(c1)))
            + (in1 - np.float32(c2)) * (in1 + np.float32(c2))
        ).astype(np.float32),
    )
    return _register_op("ANSCOMBE_TAIL_FACT2_ANT", spec)


def _weight_matrix():
    """[128, 3*BLOCK] fp16 band matrix: segment j (horizontal tap dx=j-1) has
    K2D[d, j]*S_PS on diagonal k-m = d (vertical tap dy=d-1)."""
    w = K2D.astype(np.float64) * S_PS   # [d, j]
    wm = np.zeros((128, 3 * BLOCK), dtype=np.float64)
    for j in range(3):
        for d in range(3):
            for m in range(BLOCK):
                wm[m + d, j * BLOCK + m] = w[d, j]
    return wm.astype(np.float16)


def _runt_weight_matrix():
    """[120, 3*96] fp16 block-banded matrix for the folded runt: out partition
    8*cg + m gets tap d from input partition 10*cg + m + d."""
    w = K2D.astype(np.float64) * S_PS
    wm = np.zeros((R_PIN, 3 * R_POUT), dtype=np.float64)
    for j in range(3):
        for cg in range(R_NG):
            for d in range(3):
                for m in range(R_M):
                    wm[R_KIN * cg + m + d, j * R_POUT + R_M * cg + m] = w[d, j]
    return wm.astype(np.float16)


# ------------------------------------------------------------- bass program
def build_nc(h_out=H_CORE, wc=WC):
    tail_op = _register_tail_op()
    recip_op = _register_recip1s_op()
    h_in = h_out + 2
    nc = bacc.Bacc(None, target_bir_lowering=False)
    # const AP for the sqrt bias (activation converts float bias to an AP)
    _bias = nc.alloc_sbuf_tensor("const-sqrt-bias", [128, 1], F32)
    nc.gpsimd.memset(_bias.ap(), SQRT_BIAS)
    nc.const_aps.aps[(F32, SQRT_BIAS)] = _bias.ap()
    nc.all_engine_barrier()

    x = nc.declare_dram_parameter("x", [h_in, wc], FP16, isOutput=False)
    wmat = nc.declare_dram_parameter("wm", [128, 3 * BLOCK], FP16, isOutput=False)
    wmat2 = nc.declare_dram_parameter("wm2", [R_PIN, 3 * R_POUT], FP16,
                                      isOutput=False)
    out = nc.declare_dram_parameter("out", [h_out, wc], FP16, isOutput=True)

    n_blk = 4                       # full blocks; then the folded runt
    r0_runt = n_blk * BLOCK
    n_grp = wc // GROUP
    n_pc = wc // PIECE
    SQRT = mybir.ActivationFunctionType.Sqrt
    SQUARE = mybir.ActivationFunctionType.Square

    with tile.TileContext(nc) as tc:
        with (
            tc.tile_pool(name="consts", bufs=1) as cpool,
            tc.tile_pool(name="xpool", bufs=2) as xpool,
            tc.tile_pool(name="at", bufs=2) as atpool,
            tc.tile_pool(name="runt", bufs=1) as runtpool,
            tc.tile_pool(name="rpool", bufs=2) as rpool,
            tc.tile_pool(name="opool", bufs=3) as opool,
            tc.tile_pool(name="psum", bufs=2, space="PSUM") as pspool,
        ):
            wt = cpool.tile([128, 3 * BLOCK], FP16)
            wt2 = cpool.tile([R_PIN, 3 * R_POUT], FP16)

            # block 0 uses small leading pieces so the first matmul group's
            # dependencies land ASAP; later blocks prefetch a whole block
            # ahead, so three even pieces suffice.
            PIECES0 = [0, GROUP + 6, 2 * GROUP + 6, 3 * GROUP + 6, wc]
            PIECES = [0, PIECE, 2 * PIECE, wc]

            def issue_main_input(bi):
                """DMA (column pieces, alternating queues) + border memsets."""
                r0 = bi * BLOCK
                k_in = BLOCK + 2
                xc = xpool.tile([128, wc], FP16, tag="xc")
                at = atpool.tile([128, wc + 6], FP16, tag="at")
                bounds = PIECES0 if bi == 0 else PIECES
                for k in range(len(bounds) - 1):
                    c0, c1 = bounds[k], bounds[k + 1]
                    eng = nc.sync if (bi + k) % 2 == 0 else nc.scalar
                    eng.dma_start(xc[:k_in, c0:c1], x[r0:r0 + k_in, c0:c1])
                nc.gpsimd.memset(at[:k_in, 0:3], 0.0)
                nc.gpsimd.memset(at[:k_in, wc + 3:wc + 6], 0.0)
                return xc, at

            def sqrt_piece(tiles, bi, k):
                xc, at = tiles
                bounds = PIECES0 if bi == 0 else PIECES
                c0, c1 = bounds[k], bounds[k + 1]
                nc.scalar.activation(at[:BLOCK + 2, 3 + c0:3 + c1],
                                     xc[:BLOCK + 2, c0:c1],
                                     SQRT, bias=SQRT_BIAS, scale=SQRT_SCALE)

            def issue_runt_input():
                """Folded runt input: [120, 1030] = 12 col-groups x 10 rows,
                3-col halos between groups; PAD_VAL in the outermost borders
                so sqrt maps them to exactly 0 (horizontal zero padding)."""
                x2 = runtpool.tile([R_PIN, R_COLS + 6], FP16, tag="x2")
                at2 = runtpool.tile([R_PIN, R_COLS + 6], FP16, tag="at2")
                nc.gpsimd.memset(x2[:, 0:3], PAD_VAL)
                nc.gpsimd.memset(x2[:, R_COLS + 3:R_COLS + 6], PAD_VAL)
                nc.scalar.dma_start(
                    x2[:, 3:3 + R_COLS],
                    x[r0_runt:r0_runt + R_KIN, :].rearrange(
                        "r (g c) -> g r c", g=R_NG))
                span = (R_NG - 1) * R_COLS
                nc.sync.dma_start(
                    x2[R_KIN:, 0:3],
                    x[r0_runt:r0_runt + R_KIN,
                      R_COLS - 3:R_COLS - 3 + span].rearrange(
                        "r (g c) -> g r c", g=R_NG - 1)[:, :, 0:3])
                nc.sync.dma_start(
                    x2[:(R_NG - 1) * R_KIN, R_COLS + 3:R_COLS + 6],
                    x[r0_runt:r0_runt + R_KIN,
                      R_COLS:R_COLS + span].rearrange(
                        "r (g c) -> g r c", g=R_NG - 1)[:, :, 0:3])
                return x2, at2

            def postprocess(ps, o_ap, m, width):
                r = rpool.tile([BLOCK, GROUP], F32, tag="r")
                nc.vector._custom_dve(recip_op, out=r[:m, :width],
                                      in0=ps[:m, :width],
                                      s0=RC0, s1=RC1, imm2=K3)
                nc.vector._custom_dve(tail_op, out=o_ap,
                                      in0=r[:m, :width], in1=ps[:m, :width],
                                      s0=KA, s1=KB, imm2=KQ)

            def runt_mm():
                """Folded runt conv into a psum tile; postprocess must follow
                within one pool rotation (caller emits it right after the
                surrounding group's postprocess)."""
                _, at2 = tiles[n_blk]
                ps = pspool.tile([BLOCK, GROUP], F32, tag="ps")
                for j in range(3):
                    for c0 in range(0, R_COLS, CHUNK):
                        nc.tensor.matmul(
                            ps[:R_POUT, c0:c0 + CHUNK],
                            wt2[:R_PIN, j * R_POUT:(j + 1) * R_POUT],
                            at2[:R_PIN, c0 + 3 * j:c0 + 3 * j + CHUNK],
                            start=(j == 0), stop=(j == 2),
                        )
                return ps

            def runt_post(ps):
                o2 = opool.tile([BLOCK, wc], FP16, tag="o")
                postprocess(ps, o2[:R_POUT, :R_COLS], R_POUT, R_COLS)
                # SWDGE only: the 3-level DRAM dest pattern exceeds PDMA2D
                nc.gpsimd.dma_start(
                    out[r0_runt:r0_runt + R_M, :].rearrange(
                        "r (g c) -> g r c", g=R_NG),
                    o2[:R_POUT, :R_COLS])

            # block 0 leads with a 512-col group so the first PSUM tile (and
            # the DVE) is live as early as possible
            GB0 = [0, CHUNK, GROUP] + [g * GROUP for g in range(2, n_grp + 1)]
            GB = [g * GROUP for g in range(n_grp + 1)]

            # ---- prime the pipeline (weights lead the scalar queue: the
            # first matmul needs them; block-0 piece 0 leads the sync queue)
            tiles = [None] * (n_blk + 1)
            nc.scalar.dma_start(wt[:], wmat[:])
            tiles[0] = issue_main_input(0)
            nc.scalar.dma_start(wt2[:], wmat2[:])
            sqrt_piece(tiles[0], 0, 0)
            if n_blk > 1:
                tiles[1] = issue_main_input(1)

            # ---- main blocks
            for bi in range(n_blk):
                r0 = bi * BLOCK
                k_in = BLOCK + 2
                xc, at = tiles[bi]
                last = bi == n_blk - 1
                bounds = GB0 if bi == 0 else GB
                ng = len(bounds) - 1
                o = opool.tile([BLOCK, wc], FP16, tag="o")
                if bi == 2:
                    tiles[n_blk] = issue_runt_input()
                for g in range(ng):
                    g0, g1 = bounds[g], bounds[g + 1]
                    width = g1 - g0
                    ps = pspool.tile([BLOCK, GROUP], F32, tag="ps")
                    # taps outer so consecutive matmuls share stationary weights
                    for j in range(3):
                        for c0 in range(g0, g1, CHUNK):
                            cw = min(CHUNK, g1 - c0)
                            nc.tensor.matmul(
                                ps[:BLOCK, c0 - g0:c0 - g0 + cw],
                                wt[:k_in, j * BLOCK:(j + 1) * BLOCK],
                                at[:k_in, c0 + 3 * j:c0 + 3 * j + cw],
                                start=(j == 0), stop=(j == 2),
                            )
                    if last and g == 0:
                        rps = runt_mm()
                    postprocess(ps, o[:BLOCK, g0:g1], BLOCK, width)
                    if last and g == 0:
                        runt_post(rps)
                    # software-pipelined ACT prep: finish this block's own
                    # remaining sqrt pieces first (block 0), then the next
                    # block's, so the ACT queue never delays a Square long.
                    if bi == 0 and g < 3:
                        sqrt_piece(tiles[0], 0, g + 1)
                    elif bi == 0 and g >= 4 and n_blk > 1:
                        sqrt_piece(tiles[1], 1, g - 4)
                    elif 0 < bi < n_blk - 1 and g % 2 == 0:
                        sqrt_piece(tiles[bi + 1], bi + 1, g // 2)
                    elif bi == n_blk - 2 and g == 5:
                        x2, at2 = tiles[n_blk]
                        nc.scalar.activation(at2[:, :], x2[:, :], SQRT,
                                             bias=SQRT_BIAS, scale=SQRT_SCALE)
                    if last and g == 2:
                        # block-3 cols 0:6144 -> DRAM early, split two queues
                        nc.gpsimd.dma_start(out[r0:r0 + HB, :wc // 2],
                                            o[:HB, :wc // 2])
                        nc.scalar.dma_start(out[r0 + HB:r0 + BLOCK, :wc // 2],
                                            o[HB:BLOCK, :wc // 2])
                    if last and g == 4:
                        c0, c1 = wc // 2, wc - GROUP
                        nc.gpsimd.dma_start(out[r0:r0 + HB, c0:c1],
                                            o[:HB, c0:c1])
                        nc.sync.dma_start(out[r0 + HB:r0 + BLOCK, c0:c1],
                                          o[HB:BLOCK, c0:c1])
                if not last:
                    # full-width output rows (24KB descriptors), two queues
                    nc.gpsimd.dma_start(out[r0:r0 + HB, :], o[:HB, :])
                    eng = nc.sync if bi % 2 == 0 else nc.scalar
                    eng.dma_start(out[r0 + HB:r0 + BLOCK, :], o[HB:BLOCK, :])
                else:
                    # only the last group remains after the final tail
                    c0 = wc - GROUP
                    nc.gpsimd.dma_start(out[r0:r0 + HB, c0:], o[:HB, c0:])
                    nc.sync.dma_start(out[r0 + HB:r0 + BLOCK, c0:],
                                      o[HB:BLOCK, c0:])
                if bi + 2 < n_blk:
                    tiles[bi + 2] = issue_main_input(bi + 2)
    nc.compile()
    return nc


# ------------------------------------------------------------------- driver
_CACHE = {}


def _get_nc(h_out, wc):
    key = (h_out, wc)
    if key not in _CACHE:
        _CACHE[key] = build_nc(h_out, wc)
    return _CACHE[key]


def run_sharded(x2d, n_cores=N_CORES, trace=False, **kw):
    """x2d: [H, W*C] fp32 full image (2D). Returns ([H, W*C] fp32, results)."""
    h, wc = x2d.shape
    h_core = h // n_cores
    nc = _get_nc(h_core, wc)
    wm = _weight_matrix()
    wm2 = _runt_weight_matrix()
    in_maps = []
    for i in range(n_cores):
        lo, hi = i * h_core - 1, (i + 1) * h_core + 1
        src_lo, src_hi = max(lo, 0), min(hi, h)
        if lo < 0 or hi > h:
            slab = np.full((h_core + 2, wc), PAD_VAL, dtype=np.float16)
        else:
            slab = np.empty((h_core + 2, wc), dtype=np.float16)
        slab[src_lo - lo:src_hi - lo] = x2d[src_lo:src_hi]
        in_maps.append({"x": slab, "wm": wm, "wm2": wm2})
    res = run_bass_kernel_spmd(nc, in_maps, list(range(n_cores)), trace=trace, **kw)
    full = np.concatenate([res.results[i]["out"] for i in range(n_cores)],
                          axis=0).astype(np.float32)
    return full, res


def kernel(im: np.ndarray) -> np.ndarray:
    x2d = np.asarray(im, dtype=np.float32).reshape(H, WC)
    full, _ = run_sharded(x2d)
    return full.reshape(H, W, CH)
